# revision 13
# baseline (speedup 1.0000x reference)
"""BdG gap-equation forward + analytic Jacobian on Trainium2.

Strategy
--------
Per batch matrix (8 matrices -> 8 NeuronCores, pure data parallel):

host (f64):  scatter delta blocks, eigh, t = tanh(beta*L/2),
             W[m,n] = mask*(t[n]-t[m])/(L[n]-L[m])  (tanh divided difference),
             f (gap equation), term3 (diagonal dE term)  -- all tiny.

device (f32): the O(M^2 N^2) Jacobian contraction
             term12[i,j] = sum_{m,n} G'[i,m,n] * Mf[j,m,n]
  where      Mf[j]  = conj(q0_j)xq3_j - conj(q1_j)xq2_j
                    + conj(q3_j)xq0_j - conj(q2_j)xq1_j      (rank-8 real)
             G'[i]  = (u_i x conj(v_i)) .* W                 (rank-2 .* W)
  Both stacks are generated on-chip from tiny per-j/i factor vectors via
  K=8 / K=2 TensorEngine outer-product matmuls (PSUM), fixed up / copied
  to SBUF by DVE/ACT, then contracted by a long PSUM-accumulating matmul
  chain with K = m-partitions, iterating n (the data never touches HBM).

This reformulation is algebraically exact vs the reference einsum chain
(term1+term2 collapse via C[j,n,m] = -conj(C[j,m,n])) and better
conditioned: the divided difference (t[n]-t[m])/(L[n]-L[m]) is bounded by
beta/2 while the reference's bare 1/(L[n]-L[m]) is not.
"""

import numpy as np
from contextlib import ExitStack

import concourse.bass as bass
import concourse.tile as tile
from concourse import bacc, mybir
from concourse.bass_utils import run_bass_kernel_spmd

# problem constants (hardcoded per spec: B=8, NS=48, M=48, N=192, idx=arange)
B = 8
M = 48
N = 192
EPS = 1e-10
F32 = mybir.dt.float32

MTILES = [(0, 128), (128, 64)]   # m-dim partition tiles
NCHUNKS = [(0, 96), (96, 96)]    # n-dim chunks (SBUF capacity)


def _emit(ctx: ExitStack, tc: "tile.TileContext", out96, ins):
    nc = tc.nc
    singles = ctx.enter_context(tc.tile_pool(name="singles", bufs=1))
    psum_gen = ctx.enter_context(tc.tile_pool(name="psum_gen", bufs=4, space="PSUM"))
    psum_out = ctx.enter_context(tc.tile_pool(name="psum_out", bufs=2, space="PSUM"))

    # --- load factor tensors + W ---
    # Factors live in [128p, M//4, N] tiles: j's K-row block sits at
    # partition base 32*(j%4), free index j//4. The 32-aligned bases give
    # each j a distinct PE row-group, so 4 gen-matmuls run concurrently
    # (tile_position row packing), and per-partition SBUF cost stays low.
    def load(name, kdim):
        t = singles.tile([128, M // 4, N], F32, tag=name, name=name)
        for a in range(4):
            nc.sync.dma_start(
                out=t[32 * a : 32 * a + kdim, :, :],
                in_=ins[name][:, a::4, :],
            )
        return t

    mf_stat = load("mf_stat", 8)
    mf_sre = load("mf_sre", 8)
    mf_sim = load("mf_sim", 8)
    gp_stat = load("gp_stat", 2)
    gp_sre = load("gp_sre", 2)
    gp_sim = load("gp_sim", 2)

    wt = []
    for mt, (m0, mw) in enumerate(MTILES):
        w = singles.tile([mw, N], F32, tag=f"w{mt}", name=f"w{mt}")
        nc.sync.dma_start(out=w, in_=ins["wmat"][m0 : m0 + mw, :])
        wt.append(w)

    # --- persistent stacks (overwritten each n-chunk) ---
    CS = [singles.tile([mw, 96, 96], F32, tag=f"cs{k}", name=f"cs{k}")
          for k, (m0, mw) in enumerate(MTILES)]
    GS = [singles.tile([mw, 96, 96], F32, tag=f"gs{k}", name=f"gs{k}")
          for k, (m0, mw) in enumerate(MTILES)]

    out_sb = singles.tile([96, 96], F32, tag="out_sb", name="out_sb")
    out_ps = []

    for ci, (n0, nw) in enumerate(NCHUNKS):
        # ---- generate Mf (-> CS, plain copy on ACT) and G' (-> GS, .*W on DVE)
        for stat, sre, sim_, dst, mulw in (
            (mf_stat, mf_sre, mf_sim, CS, False),
            (gp_stat, gp_sre, gp_sim, GS, True),
        ):
            kdim = 8 if stat is mf_stat else 2
            for j in range(M):
                a, jj = j % 4, j // 4
                p0 = 32 * a
                for mt, (m0, mw) in enumerate(MTILES):
                    pt = psum_gen.tile([128, 192], F32, tag="gen", name="pt")
                    nc.tensor.matmul(
                        pt[:mw, 0:nw],
                        stat[p0 : p0 + kdim, jj, m0 : m0 + mw],
                        sre[p0 : p0 + kdim, jj, n0 : n0 + nw],
                        start=True, stop=True,
                        tile_position=(p0, 0),
                    )
                    nc.tensor.matmul(
                        pt[:mw, 96 : 96 + nw],
                        stat[p0 : p0 + kdim, jj, m0 : m0 + mw],
                        sim_[p0 : p0 + kdim, jj, n0 : n0 + nw],
                        start=True, stop=True,
                        tile_position=(p0, 0),
                    )
                    if mulw:
                        nc.vector.tensor_mul(
                            dst[mt][:, :, j], pt[:mw, 0:nw], wt[mt][:, n0 : n0 + nw]
                        )
                        nc.vector.tensor_mul(
                            dst[mt][:, :, M + j], pt[:mw, 96 : 96 + nw], wt[mt][:, n0 : n0 + nw]
                        )
                    else:
                        nc.scalar.copy(dst[mt][:, :, j], pt[:mw, 0:nw])
                        nc.scalar.copy(dst[mt][:, :, M + j], pt[:mw, 96 : 96 + nw])

        # ---- main contraction for this chunk: accumulate over (n, mtile)
        po = psum_out.tile([96, 96], F32, tag="out", name="po")
        out_ps.append(po)
        nmm = 2 * nw
        k = 0
        for n in range(nw):
            for mt, (m0, mw) in enumerate(MTILES):
                nc.tensor.matmul(
                    po, GS[mt][:, n, :], CS[mt][:, n, :],
                    start=(k == 0), stop=(k == nmm - 1),
                )
                k += 1

    nc.scalar.copy(out_sb, out_ps[0])
    nc.vector.tensor_add(out_sb, out_sb, out_ps[1])
    nc.sync.dma_start(out=out96, in_=out_sb)


_NC = None


def _build_nc():
    global _NC
    if _NC is not None:
        return _NC
    nc = bacc.Bacc("TRN2", target_bir_lowering=False, debug=False)
    ins = {}
    for name, shape in [
        ("mf_stat", [8, M, N]), ("mf_sre", [8, M, N]), ("mf_sim", [8, M, N]),
        ("gp_stat", [2, M, N]), ("gp_sre", [2, M, N]), ("gp_sim", [2, M, N]),
        ("wmat", [N, N]),
    ]:
        ins[name] = nc.dram_tensor(name, shape, F32, kind="ExternalInput").ap()
    out96 = nc.dram_tensor("out96", [96, 96], F32, kind="ExternalOutput").ap()
    with tile.TileContext(nc) as tc:
        with ExitStack() as ctx:
            _emit(ctx, tc, out96, ins)
    nc.compile()
    _NC = nc
    return nc


def _host_prep(x, base_re, base_im, beta, idx, pot):
    """f64 host work: scatter, eigh, small terms; returns per-core in_maps
    plus everything needed for final assembly."""
    x = np.asarray(x, np.float64)
    base = np.asarray(base_re, np.float64) + 1j * np.asarray(base_im, np.float64)
    beta = float(np.asarray(beta).reshape(-1)[0])
    idx = np.asarray(idx).astype(np.int64)
    pot = np.asarray(pot, np.float64)

    Bn, Mn = x.shape
    Nn = base.shape[-1]

    JSIG = np.array([[0.0, 1.0], [-1.0, 0.0]], dtype=np.complex128)
    rows = 4 * idx[:, None] + np.arange(2)      # [M,2]
    cols = rows + 2
    H = base.copy()
    top = x[:, :, None, None].astype(np.complex128) * JSIG  # [B,M,2,2]
    bot = np.conj(np.swapaxes(top, -1, -2))
    bi = np.arange(Bn)[:, None, None, None]
    H[bi, rows[None, :, :, None], cols[None, :, None, :]] = top[:, :, :, :]
    H[bi, cols[None, :, :, None], rows[None, :, None, :]] = bot[:, :, :, :]

    L, Q = np.linalg.eigh(H)                    # [B,N], [B,N,N]

    t = np.tanh(0.5 * beta * L)
    dt = 0.5 * beta * (1.0 - t * t)
    q0 = Q[:, 4 * idx + 0, :]
    q1 = Q[:, 4 * idx + 1, :]
    q2 = Q[:, 4 * idx + 2, :]
    q3 = Q[:, 4 * idx + 3, :]
    u, v = q0, q3

    # f (gap equation)
    f = 0.5 * pot[None, :] * np.sum(u * np.conj(v) * t[:, None, :], axis=-1)

    # W: masked tanh divided difference
    D = L[:, None, :] - L[:, :, None]           # D[m,n] = L[n]-L[m]
    mask = np.abs(D) > EPS
    W = np.where(mask, (t[:, None, :] - t[:, :, None]) / np.where(mask, D, 1.0), 0.0)

    # term3 via diag of Mf: dE[j,n] = 2*Re(conj(q0)q3 - conj(q1)q2)[j,n]
    dE = 2.0 * (np.conj(q0) * q3 - np.conj(q1) * q2).real
    y = u * np.conj(v) * dt[:, None, :]
    term3 = np.einsum("Bin,Bjn->Bij", y, dE)

    # per-core device factor tensors (f32)
    in_maps = []
    for b in range(Bn):
        r = lambda a: np.ascontiguousarray(a.real, np.float32)
        im = lambda a: np.ascontiguousarray(a.imag, np.float32)
        Q0, Q1, Q2, Q3 = q0[b], q1[b], q2[b], q3[b]
        mf_stat = np.stack([r(Q0), r(Q1), r(Q3), r(Q2), im(Q0), im(Q1), im(Q3), im(Q2)])
        mf_sre = np.stack([r(Q3), -r(Q2), r(Q0), -r(Q1), im(Q3), -im(Q2), im(Q0), -im(Q1)])
        mf_sim = np.stack([im(Q3), -im(Q2), im(Q0), -im(Q1), -r(Q3), r(Q2), -r(Q0), r(Q1)])
        gp_stat = np.stack([r(Q0), im(Q0)])
        gp_sre = np.stack([r(Q3), im(Q3)])
        gp_sim = np.stack([-im(Q3), r(Q3)])
        in_maps.append({
            "mf_stat": np.ascontiguousarray(mf_stat),
            "mf_sre": np.ascontiguousarray(mf_sre),
            "mf_sim": np.ascontiguousarray(mf_sim),
            "gp_stat": np.ascontiguousarray(gp_stat),
            "gp_sre": np.ascontiguousarray(gp_sre),
            "gp_sim": np.ascontiguousarray(gp_sim),
            "wmat": np.ascontiguousarray(W[b], dtype=np.float32).astype(np.float32),
        })

    return dict(x=x, pot=pot, f=f, term3=term3, in_maps=in_maps, Bn=Bn, Mn=Mn)


def _assemble(prep, out96_list):
    """Combine device term12 blocks with host terms into (f-x, J-I)."""
    Bn, Mn = prep["Bn"], prep["Mn"]
    term12 = np.empty((Bn, Mn, Mn), dtype=np.complex128)
    for b in range(Bn):
        o = np.asarray(out96_list[b], np.float64)
        re = o[:Mn, :Mn] - o[Mn:, Mn:]
        imp = o[:Mn, Mn:] + o[Mn:, :Mn]
        term12[b] = re + 1j * imp
    J = 0.5 * prep["pot"][None, :, None] * (term12 + prep["term3"])
    out_f = prep["f"] - prep["x"]
    out_J = J - np.eye(Mn, dtype=np.complex128)[None]
    return out_f, out_J


_LAST_RES = None


def kernel(x, base_re, base_im, beta, idx, pot):
    global _LAST_RES
    prep = _host_prep(x, base_re, base_im, beta, idx, pot)
    assert prep["Bn"] == B and prep["Mn"] == M

    nc = _build_nc()
    res = run_bass_kernel_spmd(nc, prep["in_maps"], core_ids=list(range(B)))
    _LAST_RES = res
    out96_list = [res.results[b]["out96"] for b in range(B)]
    return _assemble(prep, out96_list)


# revision 15
# speedup vs baseline: 1.5375x; 1.5375x over previous
"""BdG gap-equation forward + analytic Jacobian on Trainium2.

Strategy
--------
Per batch matrix (8 matrices -> 8 NeuronCores, pure data parallel):

host (f64):  scatter delta blocks, eigh, t = tanh(beta*L/2),
             W[m,n] = mask*(t[n]-t[m])/(L[n]-L[m])  (tanh divided difference),
             f (gap equation), term3 (diagonal dE term)  -- all tiny.

device (f32): the O(M^2 N^2) Jacobian contraction
             term12[i,j] = sum_{m,n} G'[i,m,n] * Mf[j,m,n]
  where      Mf[j]  = conj(q0_j)xq3_j - conj(q1_j)xq2_j
                    + conj(q3_j)xq0_j - conj(q2_j)xq1_j      (rank-8 real)
             G'[i]  = (u_i x conj(v_i)) .* W                 (rank-2 .* W)
  Both stacks are generated on-chip from tiny per-j/i factor vectors via
  K=8 / K=2 TensorEngine outer-product matmuls (PSUM), fixed up / copied
  to SBUF by DVE/ACT, then contracted by a long PSUM-accumulating matmul
  chain with K = m-partitions, iterating n (the data never touches HBM).

This reformulation is algebraically exact vs the reference einsum chain
(term1+term2 collapse via C[j,n,m] = -conj(C[j,m,n])) and better
conditioned: the divided difference (t[n]-t[m])/(L[n]-L[m]) is bounded by
beta/2 while the reference's bare 1/(L[n]-L[m]) is not.
"""

import numpy as np
from contextlib import ExitStack

import concourse.bass as bass
import concourse.tile as tile
from concourse import bacc, mybir
from concourse.bass_utils import run_bass_kernel_spmd

# problem constants (hardcoded per spec: B=8, NS=48, M=48, N=192, idx=arange)
B = 8
M = 48
N = 192
EPS = 1e-10
F32 = mybir.dt.float32

MTILES = [(0, 128), (128, 64)]   # m-dim partition tiles
NCHUNKS = [(0, 96), (96, 96)]    # n-dim chunks (SBUF capacity)


def _emit(ctx: ExitStack, tc: "tile.TileContext", out96, ins):
    nc = tc.nc
    singles = ctx.enter_context(tc.tile_pool(name="singles", bufs=1))
    psum_gen = ctx.enter_context(tc.tile_pool(name="psum_gen", bufs=4, space="PSUM"))
    psum_out = ctx.enter_context(tc.tile_pool(name="psum_out", bufs=2, space="PSUM"))

    # --- load factor tensors + W ---
    # Factors live in [128p, M//4, N] tiles: j's K-row block sits at
    # partition base 32*(j%4), free index j//4. The 32-aligned bases give
    # each j a distinct PE row-group, so 4 gen-matmuls run concurrently
    # (tile_position row packing), and per-partition SBUF cost stays low.
    def load(name, kdim):
        t = singles.tile([128, M // 4, N], F32, tag=name, name=name)
        for a in range(4):
            nc.sync.dma_start(
                out=t[32 * a : 32 * a + kdim, :, :],
                in_=ins[name][:, a::4, :],
            )
        return t

    mf_stat = load("mf_stat", 8)
    mf_sre = load("mf_sre", 8)
    mf_sim = load("mf_sim", 8)
    gp_stat = load("gp_stat", 2)
    gp_sre = load("gp_sre", 2)
    gp_sim = load("gp_sim", 2)

    wt = []
    for mt, (m0, mw) in enumerate(MTILES):
        w = singles.tile([mw, N], F32, tag=f"w{mt}", name=f"w{mt}")
        nc.sync.dma_start(out=w, in_=ins["wmat"][m0 : m0 + mw, :])
        wt.append(w)

    # --- persistent stacks (overwritten each n-chunk) ---
    CS = [singles.tile([mw, 96, 96], F32, tag=f"cs{k}", name=f"cs{k}")
          for k, (m0, mw) in enumerate(MTILES)]
    GS = [singles.tile([mw, 96, 96], F32, tag=f"gs{k}", name=f"gs{k}")
          for k, (m0, mw) in enumerate(MTILES)]

    out_sb = singles.tile([96, 96], F32, tag="out_sb", name="out_sb")
    out_ps = []

    for ci, (n0, nw) in enumerate(NCHUNKS):
        # ---- generate Mf (-> CS, plain copy on ACT) and G' (-> GS, .*W on DVE)
        for stat, sre, sim_, dst, mulw in (
            (mf_stat, mf_sre, mf_sim, CS, False),
            (gp_stat, gp_sre, gp_sim, GS, True),
        ):
            kdim = 8 if stat is mf_stat else 2
            for j in range(M):
                a, jj = j % 4, j // 4
                p0 = 32 * a
                for mt, (m0, mw) in enumerate(MTILES):
                    pt = psum_gen.tile([128, 192], F32, tag="gen", name="pt")
                    nc.tensor.matmul(
                        pt[:mw, 0:nw],
                        stat[p0 : p0 + kdim, jj, m0 : m0 + mw],
                        sre[p0 : p0 + kdim, jj, n0 : n0 + nw],
                        start=True, stop=True,
                        tile_position=(p0, 0),
                    )
                    nc.tensor.matmul(
                        pt[:mw, 96 : 96 + nw],
                        stat[p0 : p0 + kdim, jj, m0 : m0 + mw],
                        sim_[p0 : p0 + kdim, jj, n0 : n0 + nw],
                        start=True, stop=True,
                        tile_position=(p0, 0),
                    )
                    if mulw:
                        nc.vector.tensor_mul(
                            dst[mt][:, :, j], pt[:mw, 0:nw], wt[mt][:, n0 : n0 + nw]
                        )
                        nc.vector.tensor_mul(
                            dst[mt][:, :, M + j], pt[:mw, 96 : 96 + nw], wt[mt][:, n0 : n0 + nw]
                        )
                    else:
                        nc.scalar.copy(dst[mt][:, :, j], pt[:mw, 0:nw])
                        nc.scalar.copy(dst[mt][:, :, M + j], pt[:mw, 96 : 96 + nw])

        # ---- main contraction for this chunk: accumulate over (n, mtile)
        po = psum_out.tile([96, 96], F32, tag="out", name="po")
        out_ps.append(po)
        nmm = 2 * nw
        k = 0
        for n in range(nw):
            for mt, (m0, mw) in enumerate(MTILES):
                nc.tensor.matmul(
                    po, GS[mt][:, n, :], CS[mt][:, n, :],
                    start=(k == 0), stop=(k == nmm - 1),
                )
                k += 1

    nc.scalar.copy(out_sb, out_ps[0])
    nc.vector.tensor_add(out_sb, out_sb, out_ps[1])
    nc.sync.dma_start(out=out96, in_=out_sb)


_NC = None


def _build_nc():
    global _NC
    if _NC is not None:
        return _NC
    nc = bacc.Bacc("TRN2", target_bir_lowering=False, debug=False)
    ins = {}
    for name, shape in [
        ("mf_stat", [8, M, N]), ("mf_sre", [8, M, N]), ("mf_sim", [8, M, N]),
        ("gp_stat", [2, M, N]), ("gp_sre", [2, M, N]), ("gp_sim", [2, M, N]),
        ("wmat", [N, N]),
    ]:
        ins[name] = nc.dram_tensor(name, shape, F32, kind="ExternalInput").ap()
    out96 = nc.dram_tensor("out96", [96, 96], F32, kind="ExternalOutput").ap()
    with tile.TileContext(nc) as tc:
        with ExitStack() as ctx:
            _emit(ctx, tc, out96, ins)
    nc.compile()
    _NC = nc
    return nc


def _host_prep(x, base_re, base_im, beta, idx, pot):
    """f64 host work: scatter, eigh, small terms; returns per-core in_maps
    plus everything needed for final assembly."""
    x = np.asarray(x, np.float64)
    base = np.asarray(base_re, np.float64) + 1j * np.asarray(base_im, np.float64)
    beta = float(np.asarray(beta).reshape(-1)[0])
    idx = np.asarray(idx).astype(np.int64)
    pot = np.asarray(pot, np.float64)

    Bn, Mn = x.shape
    Nn = base.shape[-1]

    JSIG = np.array([[0.0, 1.0], [-1.0, 0.0]], dtype=np.complex128)
    rows = 4 * idx[:, None] + np.arange(2)      # [M,2]
    cols = rows + 2
    H = base.copy()
    top = x[:, :, None, None].astype(np.complex128) * JSIG  # [B,M,2,2]
    bot = np.conj(np.swapaxes(top, -1, -2))
    bi = np.arange(Bn)[:, None, None, None]
    H[bi, rows[None, :, :, None], cols[None, :, None, :]] = top[:, :, :, :]
    H[bi, cols[None, :, :, None], rows[None, :, None, :]] = bot[:, :, :, :]

    L, Q = np.linalg.eigh(H)                    # [B,N], [B,N,N]

    t = np.tanh(0.5 * beta * L)
    dt = 0.5 * beta * (1.0 - t * t)
    q0 = Q[:, 4 * idx + 0, :]
    q1 = Q[:, 4 * idx + 1, :]
    q2 = Q[:, 4 * idx + 2, :]
    q3 = Q[:, 4 * idx + 3, :]
    u, v = q0, q3

    # f (gap equation)
    f = 0.5 * pot[None, :] * np.sum(u * np.conj(v) * t[:, None, :], axis=-1)

    # W: masked tanh divided difference
    D = L[:, None, :] - L[:, :, None]           # D[m,n] = L[n]-L[m]
    mask = np.abs(D) > EPS
    W = np.where(mask, (t[:, None, :] - t[:, :, None]) / np.where(mask, D, 1.0), 0.0)

    # term3 via diag of Mf: dE[j,n] = 2*Re(conj(q0)q3 - conj(q1)q2)[j,n]
    dE = 2.0 * (np.conj(q0) * q3 - np.conj(q1) * q2).real
    y = u * np.conj(v) * dt[:, None, :]
    term3 = np.einsum("Bin,Bjn->Bij", y, dE)

    # per-core device factor tensors (f32)
    in_maps = []
    for b in range(Bn):
        r = lambda a: np.ascontiguousarray(a.real, np.float32)
        im = lambda a: np.ascontiguousarray(a.imag, np.float32)
        Q0, Q1, Q2, Q3 = q0[b], q1[b], q2[b], q3[b]
        mf_stat = np.stack([r(Q0), r(Q1), r(Q3), r(Q2), im(Q0), im(Q1), im(Q3), im(Q2)])
        mf_sre = np.stack([r(Q3), -r(Q2), r(Q0), -r(Q1), im(Q3), -im(Q2), im(Q0), -im(Q1)])
        mf_sim = np.stack([im(Q3), -im(Q2), im(Q0), -im(Q1), -r(Q3), r(Q2), -r(Q0), r(Q1)])
        gp_stat = np.stack([r(Q0), im(Q0)])
        gp_sre = np.stack([r(Q3), im(Q3)])
        gp_sim = np.stack([-im(Q3), r(Q3)])
        in_maps.append({
            "mf_stat": np.ascontiguousarray(mf_stat),
            "mf_sre": np.ascontiguousarray(mf_sre),
            "mf_sim": np.ascontiguousarray(mf_sim),
            "gp_stat": np.ascontiguousarray(gp_stat),
            "gp_sre": np.ascontiguousarray(gp_sre),
            "gp_sim": np.ascontiguousarray(gp_sim),
            "wmat": np.ascontiguousarray(W[b], dtype=np.float32).astype(np.float32),
        })

    return dict(x=x, pot=pot, f=f, term3=term3, in_maps=in_maps, Bn=Bn, Mn=Mn)


def _assemble(prep, out96_list):
    """Combine device term12 blocks with host terms into (f-x, J-I)."""
    Bn, Mn = prep["Bn"], prep["Mn"]
    term12 = np.empty((Bn, Mn, Mn), dtype=np.complex128)
    for b in range(Bn):
        o = np.asarray(out96_list[b], np.float64)
        re = o[:Mn, :Mn] - o[Mn:, Mn:]
        imp = o[:Mn, Mn:] + o[Mn:, :Mn]
        term12[b] = re + 1j * imp
    J = 0.5 * prep["pot"][None, :, None] * (term12 + prep["term3"])
    out_f = prep["f"] - prep["x"]
    out_J = J - np.eye(Mn, dtype=np.complex128)[None]
    return out_f, out_J


_LAST_RES = None
_RUNNER = None


def _make_runner(nc):
    """Cached replica of bass2jax.run_bass_via_pjrt's multi-core path: build
    the jitted shard_map once and reuse it, so warm calls skip re-tracing."""
    import jax
    import numpy as _np
    from jax.sharding import Mesh, PartitionSpec
    from jax.experimental.shard_map import shard_map
    from concourse import bass2jax, mybir as _mybir

    bass2jax.install_neuronx_cc_hook()
    assert nc.dbg_addr is None
    partition_name = nc.partition_id_tensor.name if nc.partition_id_tensor else None

    in_names, out_names, out_avals, zero_shapes = [], [], [], []
    for alloc in nc.m.functions[0].allocations:
        if not isinstance(alloc, _mybir.MemoryLocationSet):
            continue
        name = alloc.memorylocations[0].name
        if alloc.kind == "ExternalInput":
            if name != partition_name:
                in_names.append(name)
        elif alloc.kind == "ExternalOutput":
            shape = tuple(alloc.tensor_shape)
            dtype = _mybir.dt.np(alloc.dtype)
            out_names.append(name)
            out_avals.append(jax.core.ShapedArray(shape, dtype))
            zero_shapes.append((shape, dtype))
    n_params, n_outs = len(in_names), len(out_avals)
    all_names = in_names + out_names
    if partition_name is not None:
        all_names = all_names + [partition_name]

    def _body(*args):
        operands = list(args)
        if partition_name is not None:
            operands.append(bass2jax.partition_id_tensor())
        outs = bass2jax._bass_exec_p.bind(
            *operands,
            out_avals=tuple(out_avals),
            in_names=tuple(all_names),
            out_names=tuple(out_names),
            lowering_input_output_aliases=(),
            sim_require_finite=True,
            sim_require_nnan=True,
            nc=nc,
        )
        return tuple(outs)

    devices = jax.devices()[:B]
    mesh = Mesh(_np.asarray(devices), ("core",))
    specs = (PartitionSpec("core"),) * (n_params + n_outs)
    sharded = jax.jit(
        shard_map(_body, mesh=mesh, in_specs=specs,
                  out_specs=(PartitionSpec("core"),) * n_outs, check_rep=False),
        donate_argnums=tuple(range(n_params, n_params + n_outs)),
        keep_unused=True,
    )

    def run(in_maps):
        concat_in = [
            _np.concatenate([_np.asarray(in_maps[c][nm]) for c in range(B)], axis=0)
            for nm in in_names
        ]
        concat_zeros = [
            _np.zeros((B * s[0], *s[1:]), dt) for (s, dt) in zero_shapes
        ]
        out_arrs = sharded(*concat_in, *concat_zeros)
        return [
            {nm: _np.asarray(out_arrs[i]).reshape(B, *out_avals[i].shape)[c]
             for i, nm in enumerate(out_names)}
            for c in range(B)
        ]

    return run


def kernel(x, base_re, base_im, beta, idx, pot):
    global _LAST_RES, _RUNNER
    prep = _host_prep(x, base_re, base_im, beta, idx, pot)
    assert prep["Bn"] == B and prep["Mn"] == M

    nc = _build_nc()
    if _RUNNER is None:
        _RUNNER = _make_runner(nc)
    results = _RUNNER(prep["in_maps"])
    out96_list = [results[b]["out96"] for b in range(B)]
    return _assemble(prep, out96_list)


# revision 28
# speedup vs baseline: 1.5697x; 1.0209x over previous
"""BdG gap-equation forward + analytic Jacobian on Trainium2.

Strategy
--------
Per batch matrix (8 matrices -> 8 NeuronCores, pure data parallel):

host (f64):  scatter delta blocks, eigh, t = tanh(beta*L/2),
             W[m,n] = mask*(t[n]-t[m])/(L[n]-L[m])  (tanh divided difference),
             f (gap equation), term3 (diagonal dE term)  -- all tiny.

device (f32): the O(M^2 N^2) Jacobian contraction
             term12[i,j] = sum_{m,n} G'[i,m,n] * Mf[j,m,n]
  where      Mf[j]  = conj(q0_j)xq3_j - conj(q1_j)xq2_j
                    + conj(q3_j)xq0_j - conj(q2_j)xq1_j      (rank-8 real)
             G'[i]  = (u_i x conj(v_i)) .* W                 (rank-2 .* W)
  Both stacks are generated on-chip from tiny per-j/i factor vectors via
  K=8 / K=2 TensorEngine outer-product matmuls (PSUM), fixed up / copied
  to SBUF by DVE/ACT, then contracted by a long PSUM-accumulating matmul
  chain with K = m-partitions, iterating n (the data never touches HBM).

This reformulation is algebraically exact vs the reference einsum chain
(term1+term2 collapse via C[j,n,m] = -conj(C[j,m,n])) and better
conditioned: the divided difference (t[n]-t[m])/(L[n]-L[m]) is bounded by
beta/2 while the reference's bare 1/(L[n]-L[m]) is not.
"""

import numpy as np
from contextlib import ExitStack

import concourse.bass as bass
import concourse.tile as tile
from concourse import bacc, mybir
from concourse.bass_utils import run_bass_kernel_spmd

# problem constants (hardcoded per spec: B=8, NS=48, M=48, N=192, idx=arange)
B = 8
M = 48
N = 192
EPS = 1e-10
F32 = mybir.dt.float32
F16 = mybir.dt.float16
F32R = mybir.dt.float32r

MTILES = [(0, 128), (128, 64)]   # m-dim partition tiles


def _emit(ctx: ExitStack, tc: "tile.TileContext", out96, ins, parts=("gen", "copy", "main")):
    nc = tc.nc
    singles = ctx.enter_context(tc.tile_pool(name="singles", bufs=1))
    stackp = ctx.enter_context(tc.tile_pool(name="stackp", bufs=1))
    psum_gen = ctx.enter_context(tc.tile_pool(name="psum_gen", bufs=4, space="PSUM"))
    psum_out = ctx.enter_context(tc.tile_pool(name="psum_out", bufs=2, space="PSUM"))

    # --- load factor tensors + W ---
    # Factors live in [128p, M//4, cols] tiles: j's K-row block sits at
    # partition base 32*(j%4), free index j//4. The 32-aligned bases give
    # each j a distinct PE row-group, so 4 gen-matmuls run concurrently
    # (tile_position row packing), and per-partition SBUF cost stays low.
    def load(name, kdim, cols):
        t = singles.tile([128, M // 4, cols], F32R, tag=name, name=name)
        for a in range(4):
            nc.sync.dma_start(
                out=t[32 * a : 32 * a + kdim, :, :],
                in_=ins[name][:, a::4, :],
            )
        return t

    mf_stat = load("mf_stat", 8, N)
    mf_strm = load("mf_strm", 8, 2 * N)
    gp_stat = load("gp_stat", 2, N)
    gp_strm = load("gp_strm", 2, 2 * N)

    # W duplicated along a trailing 2-dim so one DVE op covers [re|im]
    wt = []
    for mt, (m0, mw) in enumerate(MTILES):
        w = singles.tile([mw, N, 2], F32, tag=f"w{mt}", name=f"w{mt}")
        for half in range(2):
            nc.sync.dma_start(out=w[:, :, half], in_=ins["wmat"][m0 : m0 + mw, :])
        wt.append(w)

    out_sb = singles.tile([96, 96], F32, tag="out_sb", name="out_sb")
    out_ps = []

    for mt, (m0, mw) in enumerate(MTILES):
        # fp16 stacks for this m-tile phase; same tag -> phase B reuses slots
        CS = stackp.tile([128, N, 96], F16, tag="cs", name=f"cs{mt}")
        GS = stackp.tile([128, N, 96], F16, tag="gs", name=f"gs{mt}")

        # ---- generate Mf (-> CS, plain copy on ACT) and G' (-> GS, .*W on DVE)
        # One fp32r matmul per j streams [re | im] n-factors (384 cols >= 256
        # keeps fp32r at 1 cycle/row).
        for stat, strm, dst, mulw in (
            (mf_stat, mf_strm, CS, False),
            (gp_stat, gp_strm, GS, True),
        ):
            kdim = 8 if stat is mf_stat else 2
            for j in range(M):
                a, jj = j % 4, j // 4
                p0 = 32 * a
                pt = psum_gen.tile([128, 2 * N], F32, tag="gen", name="pt")
                if "gen" in parts:
                    nc.tensor.matmul(
                        pt[:mw, :],
                        stat[p0 : p0 + kdim, jj, m0 : m0 + mw],
                        strm[p0 : p0 + kdim, jj, :],
                        start=True, stop=True,
                        tile_position=(p0, 0),
                    )
                elif "copy" in parts:
                    nc.vector.memset(pt[:mw, :], 0.0)
                if "copy" not in parts:
                    continue
                src = pt[:mw, :].rearrange("p (b n) -> p n b", b=2)
                dstv = dst[:mw, :, j :: M]  # cols {j, j+M} = re|im
                if mulw:
                    nc.vector.tensor_mul(dstv, src, wt[mt])
                else:
                    nc.scalar.copy(dstv, src)

        # ---- main contraction for this m-tile: accumulate over n (fp16)
        po = psum_out.tile([96, 96], F32, tag="out", name="po")
        out_ps.append(po)
        if "main" in parts:
            for n in range(N):
                nc.tensor.matmul(
                    po, GS[:mw, n, :], CS[:mw, n, :],
                    start=(n == 0), stop=(n == N - 1),
                )
        else:
            nc.vector.memset(po, 0.0)

    nc.scalar.copy(out_sb, out_ps[0])
    nc.vector.tensor_add(out_sb, out_sb, out_ps[1])
    nc.sync.dma_start(out=out96, in_=out_sb)


_NC = None


def _build_nc(parts=("gen", "copy", "main")):
    global _NC
    if _NC is not None and parts == ("gen", "copy", "main"):
        return _NC
    nc = bacc.Bacc("TRN2", target_bir_lowering=False, debug=False)
    ins = {}
    for name, shape in [
        ("mf_stat", [8, M, N]), ("mf_strm", [8, M, 2 * N]),
        ("gp_stat", [2, M, N]), ("gp_strm", [2, M, 2 * N]),
        ("wmat", [N, N]),
    ]:
        dt = F32 if name == "wmat" else F32R
        ins[name] = nc.dram_tensor(name, shape, dt, kind="ExternalInput").ap()
    out96 = nc.dram_tensor("out96", [96, 96], F32, kind="ExternalOutput").ap()
    with tile.TileContext(nc) as tc:
        with ExitStack() as ctx:
            _emit(ctx, tc, out96, ins)
    nc.compile()
    _NC = nc
    return nc


def _host_prep(x, base_re, base_im, beta, idx, pot):
    """f64 host work: scatter, eigh, small terms; returns per-core in_maps
    plus everything needed for final assembly."""
    x = np.asarray(x, np.float64)
    base = np.asarray(base_re, np.float64) + 1j * np.asarray(base_im, np.float64)
    beta = float(np.asarray(beta).reshape(-1)[0])
    idx = np.asarray(idx).astype(np.int64)
    pot = np.asarray(pot, np.float64)

    Bn, Mn = x.shape
    Nn = base.shape[-1]

    JSIG = np.array([[0.0, 1.0], [-1.0, 0.0]], dtype=np.complex128)
    rows = 4 * idx[:, None] + np.arange(2)      # [M,2]
    cols = rows + 2
    H = base.copy()
    top = x[:, :, None, None].astype(np.complex128) * JSIG  # [B,M,2,2]
    bot = np.conj(np.swapaxes(top, -1, -2))
    bi = np.arange(Bn)[:, None, None, None]
    H[bi, rows[None, :, :, None], cols[None, :, None, :]] = top[:, :, :, :]
    H[bi, cols[None, :, :, None], rows[None, :, None, :]] = bot[:, :, :, :]

    L, Q = np.linalg.eigh(H)                    # [B,N], [B,N,N]

    t = np.tanh(0.5 * beta * L)
    dt = 0.5 * beta * (1.0 - t * t)
    q0 = Q[:, 4 * idx + 0, :]
    q1 = Q[:, 4 * idx + 1, :]
    q2 = Q[:, 4 * idx + 2, :]
    q3 = Q[:, 4 * idx + 3, :]
    u, v = q0, q3

    # f (gap equation)
    f = 0.5 * pot[None, :] * np.sum(u * np.conj(v) * t[:, None, :], axis=-1)

    # W: masked tanh divided difference
    D = L[:, None, :] - L[:, :, None]           # D[m,n] = L[n]-L[m]
    mask = np.abs(D) > EPS
    W = np.where(mask, (t[:, None, :] - t[:, :, None]) / np.where(mask, D, 1.0), 0.0)

    # term3 via diag of Mf: dE[j,n] = 2*Re(conj(q0)q3 - conj(q1)q2)[j,n]
    dE = 2.0 * (np.conj(q0) * q3 - np.conj(q1) * q2).real
    y = u * np.conj(v) * dt[:, None, :]
    term3 = np.einsum("Bin,Bjn->Bij", y, dE)

    # per-core device factor tensors (f32)
    in_maps = []
    for b in range(Bn):
        r = lambda a: np.ascontiguousarray(a.real, np.float32)
        im = lambda a: np.ascontiguousarray(a.imag, np.float32)
        Q0, Q1, Q2, Q3 = q0[b], q1[b], q2[b], q3[b]
        mf_stat = np.stack([r(Q0), r(Q1), r(Q3), r(Q2), im(Q0), im(Q1), im(Q3), im(Q2)])
        mf_sre = np.stack([r(Q3), -r(Q2), r(Q0), -r(Q1), im(Q3), -im(Q2), im(Q0), -im(Q1)])
        mf_sim = np.stack([im(Q3), -im(Q2), im(Q0), -im(Q1), -r(Q3), r(Q2), -r(Q0), r(Q1)])
        gp_stat = np.stack([r(Q0), im(Q0)])
        gp_sre = np.stack([r(Q3), im(Q3)])
        gp_sim = np.stack([-im(Q3), r(Q3)])
        in_maps.append({
            "mf_stat": np.ascontiguousarray(mf_stat),
            "mf_strm": np.ascontiguousarray(np.concatenate([mf_sre, mf_sim], axis=-1)),
            "gp_stat": np.ascontiguousarray(gp_stat),
            "gp_strm": np.ascontiguousarray(np.concatenate([gp_sre, gp_sim], axis=-1)),
            "wmat": np.ascontiguousarray(W[b], dtype=np.float32).astype(np.float32),
        })

    return dict(x=x, pot=pot, f=f, term3=term3, in_maps=in_maps, Bn=Bn, Mn=Mn)


def _assemble(prep, out96_list):
    """Combine device term12 blocks with host terms into (f-x, J-I)."""
    Bn, Mn = prep["Bn"], prep["Mn"]
    term12 = np.empty((Bn, Mn, Mn), dtype=np.complex128)
    for b in range(Bn):
        o = np.asarray(out96_list[b], np.float64)
        re = o[:Mn, :Mn] - o[Mn:, Mn:]
        imp = o[:Mn, Mn:] + o[Mn:, :Mn]
        term12[b] = re + 1j * imp
    J = 0.5 * prep["pot"][None, :, None] * (term12 + prep["term3"])
    out_f = prep["f"] - prep["x"]
    out_J = J - np.eye(Mn, dtype=np.complex128)[None]
    return out_f, out_J


_LAST_RES = None
_RUNNER = None


def _make_runner(nc):
    """Cached replica of bass2jax.run_bass_via_pjrt's multi-core path: build
    the jitted shard_map once and reuse it, so warm calls skip re-tracing."""
    import jax
    import numpy as _np
    from jax.sharding import Mesh, PartitionSpec
    from jax.experimental.shard_map import shard_map
    from concourse import bass2jax, mybir as _mybir

    bass2jax.install_neuronx_cc_hook()
    assert nc.dbg_addr is None
    partition_name = nc.partition_id_tensor.name if nc.partition_id_tensor else None

    in_names, out_names, out_avals, zero_shapes = [], [], [], []
    for alloc in nc.m.functions[0].allocations:
        if not isinstance(alloc, _mybir.MemoryLocationSet):
            continue
        name = alloc.memorylocations[0].name
        if alloc.kind == "ExternalInput":
            if name != partition_name:
                in_names.append(name)
        elif alloc.kind == "ExternalOutput":
            shape = tuple(alloc.tensor_shape)
            dtype = _mybir.dt.np(alloc.dtype)
            out_names.append(name)
            out_avals.append(jax.core.ShapedArray(shape, dtype))
            zero_shapes.append((shape, dtype))
    n_params, n_outs = len(in_names), len(out_avals)
    all_names = in_names + out_names
    if partition_name is not None:
        all_names = all_names + [partition_name]

    def _body(*args):
        operands = list(args)
        if partition_name is not None:
            operands.append(bass2jax.partition_id_tensor())
        outs = bass2jax._bass_exec_p.bind(
            *operands,
            out_avals=tuple(out_avals),
            in_names=tuple(all_names),
            out_names=tuple(out_names),
            lowering_input_output_aliases=(),
            sim_require_finite=True,
            sim_require_nnan=True,
            nc=nc,
        )
        return tuple(outs)

    devices = jax.devices()[:B]
    mesh = Mesh(_np.asarray(devices), ("core",))
    specs = (PartitionSpec("core"),) * (n_params + n_outs)
    sharded = jax.jit(
        shard_map(_body, mesh=mesh, in_specs=specs,
                  out_specs=(PartitionSpec("core"),) * n_outs, check_rep=False),
        donate_argnums=tuple(range(n_params, n_params + n_outs)),
        keep_unused=True,
    )

    def run(in_maps):
        concat_in = [
            _np.concatenate([_np.asarray(in_maps[c][nm]) for c in range(B)], axis=0)
            for nm in in_names
        ]
        concat_zeros = [
            _np.zeros((B * s[0], *s[1:]), dt) for (s, dt) in zero_shapes
        ]
        out_arrs = sharded(*concat_in, *concat_zeros)
        return [
            {nm: _np.asarray(out_arrs[i]).reshape(B, *out_avals[i].shape)[c]
             for i, nm in enumerate(out_names)}
            for c in range(B)
        ]

    return run


def kernel(x, base_re, base_im, beta, idx, pot):
    global _LAST_RES, _RUNNER
    prep = _host_prep(x, base_re, base_im, beta, idx, pot)
    assert prep["Bn"] == B and prep["Mn"] == M

    nc = _build_nc()
    if _RUNNER is None:
        _RUNNER = _make_runner(nc)
    results = _RUNNER(prep["in_maps"])
    out96_list = [results[b]["out96"] for b in range(B)]
    return _assemble(prep, out96_list)


# revision 41
# speedup vs baseline: 16.2503x; 10.3528x over previous
"""BdG gap-equation forward + analytic Jacobian on Trainium2.

Strategy
--------
Per batch matrix (8 matrices -> 8 NeuronCores, pure data parallel):

host (f64):  scatter delta blocks, eigh, t = tanh(beta*L/2),
             W[m,n] = mask*(t[n]-t[m])/(L[n]-L[m])  (tanh divided difference),
             f (gap equation), term3 (diagonal dE term)  -- all tiny.

device (f32): the O(M^2 N^2) Jacobian contraction
             term12[i,j] = sum_{m,n} G'[i,m,n] * Mf[j,m,n]
  where      Mf[j]  = conj(q0_j)xq3_j - conj(q1_j)xq2_j
                    + conj(q3_j)xq0_j - conj(q2_j)xq1_j      (rank-8 real)
             G'[i]  = (u_i x conj(v_i)) .* W                 (rank-2 .* W)
  Both stacks are generated on-chip from tiny per-j/i factor vectors via
  K=8 / K=2 TensorEngine outer-product matmuls (PSUM), fixed up / copied
  to SBUF by DVE/ACT, then contracted by a long PSUM-accumulating matmul
  chain with K = m-partitions, iterating n (the data never touches HBM).

This reformulation is algebraically exact vs the reference einsum chain
(term1+term2 collapse via C[j,n,m] = -conj(C[j,m,n])) and better
conditioned: the divided difference (t[n]-t[m])/(L[n]-L[m]) is bounded by
beta/2 while the reference's bare 1/(L[n]-L[m]) is not.
"""

import numpy as np
from contextlib import ExitStack

import concourse.bass as bass
import concourse.tile as tile
from concourse import bacc, mybir
from concourse.bass_utils import run_bass_kernel_spmd

# problem constants (hardcoded per spec: B=8, NS=48, M=48, N=192, idx=arange)
B = 8
M = 48
N = 192
EPS = 1e-10
F32 = mybir.dt.float32
F16 = mybir.dt.float16
F32R = mybir.dt.float32r

MTILES = [(0, 128), (128, 64)]   # m-dim partition tiles


def _emit(ctx: ExitStack, tc: "tile.TileContext", out96, ins, parts=("gen", "copy", "main"), reps=1):
    nc = tc.nc
    singles = ctx.enter_context(tc.tile_pool(name="singles", bufs=1))
    stackp = ctx.enter_context(tc.tile_pool(name="stackp", bufs=1))
    psum_gen = ctx.enter_context(tc.tile_pool(name="psum_gen", bufs=4, space="PSUM"))
    psum_out = ctx.enter_context(tc.tile_pool(name="psum_out", bufs=2, space="PSUM"))

    # --- load factor tensors + W ---
    # Factors live in [128p, M//4, cols] tiles: j's K-row block sits at
    # partition base 32*(j%4), free index j//4. The 32-aligned bases give
    # each j a distinct PE row-group, so 4 gen-matmuls run concurrently
    # (tile_position row packing), and per-partition SBUF cost stays low.
    def load(name, kdim, cols):
        t = singles.tile([128, M // 4, cols], F32R, tag=name, name=name)
        nc.gpsimd.memset(t.bitcast(F32), 0.0)  # fill partition holes (SA/SAN full-tile ops)
        for a in range(4):
            nc.sync.dma_start(
                out=t[32 * a : 32 * a + kdim, :, :],
                in_=ins[name][:, a::4, :],
            )
        return t

    mf_stat = load("mf_stat", 8, N)   # unsigned q-factor rows
    sgn = singles.tile([128, 1], F32, tag="sgn", name="sgn")
    nc.sync.dma_start(out=sgn, in_=ins["sgn"])

    # Derive all gen operands on-device from mf_stat:
    #   SA  = sgn .* S   (rows [q0r,-q1r,q3r,-q2r,q0i,-q1i,q3i,-q2i])
    #   SAN = -SA
    # then streams are pure row-block permutations of SA/SAN/S via SBUF DMAs.
    S = mf_stat
    SA = singles.tile([128, M // 4, N], F32R, tag="SA", name="SA")
    SAN = singles.tile([128, M // 4, N], F32R, tag="SAN", name="SAN")
    nc.scalar.mul(SA, S, sgn)
    nc.vector.tensor_scalar_mul(SAN, SA, -1.0)

    mf_strm = singles.tile([128, M // 4, 2 * N], F32R, tag="mf_strm", name="mf_strm")
    gp_stat = singles.tile([128, M // 4, N], F32R, tag="gp_stat", name="gp_stat")
    gp_strm = singles.tile([128, M // 4, 2 * N], F32R, tag="gp_strm", name="gp_strm")
    RE_BLOCKS = [(0, SA, 2), (2, SA, 0), (4, SA, 6), (6, SA, 4)]  # (dst k, src, src k)
    IM_BLOCKS = [(0, SA, 6), (2, SA, 4), (4, SAN, 2), (6, SAN, 0)]
    for a in range(4):
        P = 32 * a
        for dk, srct, sk in RE_BLOCKS:
            nc.sync.dma_start(out=mf_strm[P + dk : P + dk + 2, :, 0:N],
                              in_=srct[P + sk : P + sk + 2, :, :])
        for dk, srct, sk in IM_BLOCKS:
            nc.sync.dma_start(out=mf_strm[P + dk : P + dk + 2, :, N : 2 * N],
                              in_=srct[P + sk : P + sk + 2, :, :])
        # gp: stationary [ur,ui] = S[0],S[4]; re [vr,vi] = S[2],S[6]; im [-vi,vr]
        nc.sync.dma_start(out=gp_stat[P : P + 1, :, :], in_=S[P : P + 1, :, :])
        nc.sync.dma_start(out=gp_stat[P + 1 : P + 2, :, :], in_=S[P + 4 : P + 5, :, :])
        nc.sync.dma_start(out=gp_strm[P : P + 1, :, 0:N], in_=S[P + 2 : P + 3, :, :])
        nc.sync.dma_start(out=gp_strm[P + 1 : P + 2, :, 0:N], in_=S[P + 6 : P + 7, :, :])
        nc.sync.dma_start(out=gp_strm[P : P + 1, :, N : 2 * N], in_=SAN[P + 6 : P + 7, :, :])
        nc.sync.dma_start(out=gp_strm[P + 1 : P + 2, :, N : 2 * N], in_=S[P + 2 : P + 3, :, :])

    # W duplicated along a trailing 2-dim so one DVE op covers [re|im]
    wt = []
    for mt, (m0, mw) in enumerate(MTILES):
        w = singles.tile([mw, N, 2], F32, tag=f"w{mt}", name=f"w{mt}")
        for half in range(2):
            nc.sync.dma_start(out=w[:, :, half], in_=ins["wmat"][m0 : m0 + mw, :])
        wt.append(w)

    out_sb = singles.tile([96, 96], F32, tag="out_sb", name="out_sb")

    for rep in range(reps):
        out_ps = _emit_compute(tc, parts, singles, stackp, psum_gen, psum_out,
                               mf_stat, mf_strm, gp_stat, gp_strm, wt)

    nc.scalar.copy(out_sb, out_ps[0])
    nc.vector.tensor_add(out_sb, out_sb, out_ps[1])
    nc.sync.dma_start(out=out96, in_=out_sb)


def _emit_compute(tc, parts, singles, stackp, psum_gen, psum_out,
                  mf_stat, mf_strm, gp_stat, gp_strm, wt):
    nc = tc.nc
    out_ps = []
    for mt, (m0, mw) in enumerate(MTILES):
        # fp16 stacks for this m-tile phase; same tag -> phase B reuses slots
        CS = stackp.tile([128, N, 96], F16, tag="cs", name=f"cs{mt}")
        GS = stackp.tile([128, N, 96], F16, tag="gs", name=f"gs{mt}")

        # ---- generate Mf (-> CS, plain copy on ACT) and G' (-> GS, .*W on DVE)
        # One fp32r matmul per j streams [re | im] n-factors (384 cols >= 256
        # keeps fp32r at 1 cycle/row).
        for stat, strm, dst, mulw in (
            (mf_stat, mf_strm, CS, False),
            (gp_stat, gp_strm, GS, True),
        ):
            kdim = 8 if stat is mf_stat else 2
            for j in range(M):
                a, jj = j % 4, j // 4
                p0 = 32 * a
                pt = psum_gen.tile([128, 2 * N], F32, tag="gen", name="pt")
                if "gen" in parts:
                    nc.tensor.matmul(
                        pt[:mw, :],
                        stat[p0 : p0 + kdim, jj, m0 : m0 + mw],
                        strm[p0 : p0 + kdim, jj, :],
                        start=True, stop=True,
                        tile_position=(p0, 0),
                    )
                elif "copy" in parts:
                    nc.vector.memset(pt[:mw, :], 0.0)
                if "copy" not in parts:
                    continue
                src = pt[:mw, :].rearrange("p (b n) -> p n b", b=2)
                dstv = dst[:mw, :, j :: M]  # cols {j, j+M} = re|im
                if mulw:
                    nc.vector.tensor_mul(dstv, src, wt[mt])
                else:
                    nc.scalar.copy(dstv, src)

        # ---- main contraction for this m-tile: accumulate over n (fp16)
        po = psum_out.tile([96, 96], F32, tag="out", name="po")
        out_ps.append(po)
        if "main" in parts:
            for n in range(N):
                nc.tensor.matmul(
                    po, GS[:mw, n, :], CS[:mw, n, :],
                    start=(n == 0), stop=(n == N - 1),
                )
        else:
            nc.vector.memset(po, 0.0)
    return out_ps


_NC = None


def _build_nc(parts=("gen", "copy", "main"), reps=1):
    global _NC
    if _NC is not None and parts == ("gen", "copy", "main") and reps == 1:
        return _NC
    nc = bacc.Bacc("TRN2", target_bir_lowering=False, debug=False)
    ins = {}
    for name, shape in [
        ("mf_stat", [8, M, N]), ("sgn", [128, 1]), ("wmat", [N, N]),
    ]:
        dt = F32R if name == "mf_stat" else F32
        ins[name] = nc.dram_tensor(name, shape, dt, kind="ExternalInput").ap()
    out96 = nc.dram_tensor("out96", [96, 96], F32, kind="ExternalOutput").ap()
    with tile.TileContext(nc) as tc:
        with ExitStack() as ctx:
            _emit(ctx, tc, out96, ins, parts=parts, reps=reps)
    nc.compile()
    if parts == ("gen", "copy", "main") and reps == 1:
        _NC = nc
    return nc


def _host_prep(x, base_re, base_im, beta, idx, pot):
    """f64 host work: scatter, eigh, small terms; returns per-core in_maps
    plus everything needed for final assembly."""
    x = np.asarray(x, np.float64)
    base = np.asarray(base_re, np.float64) + 1j * np.asarray(base_im, np.float64)
    beta = float(np.asarray(beta).reshape(-1)[0])
    idx = np.asarray(idx).astype(np.int64)
    pot = np.asarray(pot, np.float64)

    Bn, Mn = x.shape
    Nn = base.shape[-1]

    JSIG = np.array([[0.0, 1.0], [-1.0, 0.0]], dtype=np.complex128)
    rows = 4 * idx[:, None] + np.arange(2)      # [M,2]
    cols = rows + 2
    H = base.copy()
    top = x[:, :, None, None].astype(np.complex128) * JSIG  # [B,M,2,2]
    bot = np.conj(np.swapaxes(top, -1, -2))
    bi = np.arange(Bn)[:, None, None, None]
    H[bi, rows[None, :, :, None], cols[None, :, None, :]] = top[:, :, :, :]
    H[bi, cols[None, :, :, None], rows[None, :, None, :]] = bot[:, :, :, :]

    L, Q = np.linalg.eigh(H)                    # [B,N], [B,N,N]

    t = np.tanh(0.5 * beta * L)
    dt = 0.5 * beta * (1.0 - t * t)
    q0 = Q[:, 4 * idx + 0, :]
    q1 = Q[:, 4 * idx + 1, :]
    q2 = Q[:, 4 * idx + 2, :]
    q3 = Q[:, 4 * idx + 3, :]
    u, v = q0, q3

    # f (gap equation)
    f = 0.5 * pot[None, :] * np.sum(u * np.conj(v) * t[:, None, :], axis=-1)

    # W: masked tanh divided difference
    D = L[:, None, :] - L[:, :, None]           # D[m,n] = L[n]-L[m]
    mask = np.abs(D) > EPS
    W = np.where(mask, (t[:, None, :] - t[:, :, None]) / np.where(mask, D, 1.0), 0.0)

    # term3 via diag of Mf: dE[j,n] = 2*Re(conj(q0)q3 - conj(q1)q2)[j,n]
    dE = 2.0 * (np.conj(q0) * q3 - np.conj(q1) * q2).real
    y = u * np.conj(v) * dt[:, None, :]
    term3 = np.einsum("Bin,Bjn->Bij", y, dE)

    # per-core device factor tensors (f32); streams/signs derive on-device
    sgn = np.ones((128, 1), np.float32)
    for a in range(4):
        sgn[32 * a + 1 : 32 * a + 8 : 2] = -1.0
    in_maps = []
    for b in range(Bn):
        r = lambda a: np.ascontiguousarray(a.real, np.float32)
        im = lambda a: np.ascontiguousarray(a.imag, np.float32)
        Q0, Q1, Q2, Q3 = q0[b], q1[b], q2[b], q3[b]
        mf_stat = np.stack([r(Q0), r(Q1), r(Q3), r(Q2), im(Q0), im(Q1), im(Q3), im(Q2)])
        in_maps.append({
            "mf_stat": np.ascontiguousarray(mf_stat),
            "sgn": sgn,
            "wmat": np.ascontiguousarray(W[b], dtype=np.float32).astype(np.float32),
        })

    return dict(x=x, pot=pot, f=f, term3=term3, in_maps=in_maps, Bn=Bn, Mn=Mn)


def _assemble(prep, out96_list):
    """Combine device term12 blocks with host terms into (f-x, J-I)."""
    Bn, Mn = prep["Bn"], prep["Mn"]
    term12 = np.empty((Bn, Mn, Mn), dtype=np.complex128)
    for b in range(Bn):
        o = np.asarray(out96_list[b], np.float64)
        re = o[:Mn, :Mn] - o[Mn:, Mn:]
        imp = o[:Mn, Mn:] + o[Mn:, :Mn]
        term12[b] = re + 1j * imp
    J = 0.5 * prep["pot"][None, :, None] * (term12 + prep["term3"])
    out_f = prep["f"] - prep["x"]
    out_J = J - np.eye(Mn, dtype=np.complex128)[None]
    return out_f, out_J


_LAST_RES = None
_RUNNER = None


def _make_runner(nc):
    """Cached replica of bass2jax.run_bass_via_pjrt's multi-core path: build
    the jitted shard_map once and reuse it, so warm calls skip re-tracing."""
    import jax
    import numpy as _np
    from jax.sharding import Mesh, PartitionSpec
    from jax.experimental.shard_map import shard_map
    from concourse import bass2jax, mybir as _mybir

    bass2jax.install_neuronx_cc_hook()
    assert nc.dbg_addr is None
    partition_name = nc.partition_id_tensor.name if nc.partition_id_tensor else None

    in_names, out_names, out_avals, zero_shapes = [], [], [], []
    for alloc in nc.m.functions[0].allocations:
        if not isinstance(alloc, _mybir.MemoryLocationSet):
            continue
        name = alloc.memorylocations[0].name
        if alloc.kind == "ExternalInput":
            if name != partition_name:
                in_names.append(name)
        elif alloc.kind == "ExternalOutput":
            shape = tuple(alloc.tensor_shape)
            dtype = _mybir.dt.np(alloc.dtype)
            out_names.append(name)
            out_avals.append(jax.core.ShapedArray(shape, dtype))
            zero_shapes.append((shape, dtype))
    n_params, n_outs = len(in_names), len(out_avals)
    all_names = in_names + out_names
    if partition_name is not None:
        all_names = all_names + [partition_name]

    def _body(*args):
        operands = list(args)
        if partition_name is not None:
            operands.append(bass2jax.partition_id_tensor())
        outs = bass2jax._bass_exec_p.bind(
            *operands,
            out_avals=tuple(out_avals),
            in_names=tuple(all_names),
            out_names=tuple(out_names),
            lowering_input_output_aliases=(),
            sim_require_finite=True,
            sim_require_nnan=True,
            nc=nc,
        )
        return tuple(outs)

    devices = jax.devices()[:B]
    mesh = Mesh(_np.asarray(devices), ("core",))
    specs = (PartitionSpec("core"),) * (n_params + n_outs)
    sharded = jax.jit(
        shard_map(_body, mesh=mesh, in_specs=specs,
                  out_specs=(PartitionSpec("core"),) * n_outs, check_rep=False),
        donate_argnums=tuple(range(n_params, n_params + n_outs)),
        keep_unused=True,
    )

    def run(in_maps):
        concat_in = [
            _np.concatenate([_np.asarray(in_maps[c][nm]) for c in range(B)], axis=0)
            for nm in in_names
        ]
        concat_zeros = [
            _np.zeros((B * s[0], *s[1:]), dt) for (s, dt) in zero_shapes
        ]
        out_arrs = sharded(*concat_in, *concat_zeros)
        return [
            {nm: _np.asarray(out_arrs[i]).reshape(B, *out_avals[i].shape)[c]
             for i, nm in enumerate(out_names)}
            for c in range(B)
        ]

    return run


def kernel(x, base_re, base_im, beta, idx, pot):
    global _LAST_RES, _RUNNER
    prep = _host_prep(x, base_re, base_im, beta, idx, pot)
    assert prep["Bn"] == B and prep["Mn"] == M

    nc = _build_nc()
    if _RUNNER is None:
        _RUNNER = _make_runner(nc)
    results = _RUNNER(prep["in_maps"])
    out96_list = [results[b]["out96"] for b in range(B)]
    return _assemble(prep, out96_list)


# revision 44
# speedup vs baseline: 17.8513x; 1.0985x over previous
"""BdG gap-equation forward + analytic Jacobian on Trainium2.

Strategy
--------
Per batch matrix (8 matrices -> 8 NeuronCores, pure data parallel):

host (f64):  scatter delta blocks, eigh, t = tanh(beta*L/2),
             W[m,n] = mask*(t[n]-t[m])/(L[n]-L[m])  (tanh divided difference),
             f (gap equation), term3 (diagonal dE term)  -- all tiny.

device (f32): the O(M^2 N^2) Jacobian contraction
             term12[i,j] = sum_{m,n} G'[i,m,n] * Mf[j,m,n]
  where      Mf[j]  = conj(q0_j)xq3_j - conj(q1_j)xq2_j
                    + conj(q3_j)xq0_j - conj(q2_j)xq1_j      (rank-8 real)
             G'[i]  = (u_i x conj(v_i)) .* W                 (rank-2 .* W)
  Both stacks are generated on-chip from tiny per-j/i factor vectors via
  K=8 / K=2 TensorEngine outer-product matmuls (PSUM), fixed up / copied
  to SBUF by DVE/ACT, then contracted by a long PSUM-accumulating matmul
  chain with K = m-partitions, iterating n (the data never touches HBM).

This reformulation is algebraically exact vs the reference einsum chain
(term1+term2 collapse via C[j,n,m] = -conj(C[j,m,n])) and better
conditioned: the divided difference (t[n]-t[m])/(L[n]-L[m]) is bounded by
beta/2 while the reference's bare 1/(L[n]-L[m]) is not.
"""

import numpy as np
from contextlib import ExitStack

import concourse.bass as bass
import concourse.tile as tile
from concourse import bacc, mybir
from concourse.bass_utils import run_bass_kernel_spmd

# problem constants (hardcoded per spec: B=8, NS=48, M=48, N=192, idx=arange)
B = 8
M = 48
N = 192
EPS = 1e-10
F32 = mybir.dt.float32
F16 = mybir.dt.float16
F32R = mybir.dt.float32r

MTILES = [(0, 128), (128, 64)]   # m-dim partition tiles


def _emit(ctx: ExitStack, tc: "tile.TileContext", out96, ins, parts=("gen", "copy", "main"), reps=1):
    nc = tc.nc
    singles = ctx.enter_context(tc.tile_pool(name="singles", bufs=1))
    stackp = ctx.enter_context(tc.tile_pool(name="stackp", bufs=1))
    psum_gen = ctx.enter_context(tc.tile_pool(name="psum_gen", bufs=3, space="PSUM"))
    psum_out = ctx.enter_context(tc.tile_pool(name="psum_out", bufs=2, space="PSUM"))

    # --- load factor tensors + W ---
    # Factors live in [128p, M//4, cols] tiles: j's K-row block sits at
    # partition base 32*(j%4), free index j//4. The 32-aligned bases give
    # each j a distinct PE row-group, so 4 gen-matmuls run concurrently
    # (tile_position row packing), and per-partition SBUF cost stays low.
    def load(name, kdim, cols):
        t = singles.tile([128, M // 4, cols], F32R, tag=name, name=name)
        nc.gpsimd.memset(t.bitcast(F32), 0.0)  # fill partition holes (SA/SAN full-tile ops)
        for a in range(4):
            nc.sync.dma_start(
                out=t[32 * a : 32 * a + kdim, :, :],
                in_=ins[name][:, a::4, :],
            )
        return t

    mf_stat = load("mf_stat", 8, N)   # unsigned q-factor rows
    sgn = singles.tile([128, 1], F32, tag="sgn", name="sgn")
    nc.sync.dma_start(out=sgn, in_=ins["sgn"])

    # Derive all gen operands on-device from mf_stat:
    #   SA  = sgn .* S   (rows [q0r,-q1r,q3r,-q2r,q0i,-q1i,q3i,-q2i])
    #   SAN = -SA
    # then streams are pure row-block permutations of SA/SAN/S via SBUF DMAs.
    S = mf_stat
    SA = singles.tile([128, M // 4, N], F32R, tag="SA", name="SA")
    SAN = singles.tile([128, M // 4, N], F32R, tag="SAN", name="SAN")
    nc.scalar.mul(SA, S, sgn)
    nc.vector.tensor_scalar_mul(SAN, SA, -1.0)

    mf_strm = singles.tile([128, M // 4, 2 * N], F32R, tag="mf_strm", name="mf_strm")
    gp_stat = singles.tile([128, M // 4, N], F32R, tag="gp_stat", name="gp_stat")
    gp_strm = singles.tile([128, M // 4, 2 * N], F32R, tag="gp_strm", name="gp_strm")
    RE_BLOCKS = [(0, SA, 2), (2, SA, 0), (4, SA, 6), (6, SA, 4)]  # (dst k, src, src k)
    IM_BLOCKS = [(0, SA, 6), (2, SA, 4), (4, SAN, 2), (6, SAN, 0)]
    for a in range(4):
        P = 32 * a
        for dk, srct, sk in RE_BLOCKS:
            nc.sync.dma_start(out=mf_strm[P + dk : P + dk + 2, :, 0:N],
                              in_=srct[P + sk : P + sk + 2, :, :])
        for dk, srct, sk in IM_BLOCKS:
            nc.sync.dma_start(out=mf_strm[P + dk : P + dk + 2, :, N : 2 * N],
                              in_=srct[P + sk : P + sk + 2, :, :])
        # gp: stationary [ur,ui] = S[0],S[4]; re [vr,vi] = S[2],S[6]; im [-vi,vr]
        nc.sync.dma_start(out=gp_stat[P : P + 1, :, :], in_=S[P : P + 1, :, :])
        nc.sync.dma_start(out=gp_stat[P + 1 : P + 2, :, :], in_=S[P + 4 : P + 5, :, :])
        nc.sync.dma_start(out=gp_strm[P : P + 1, :, 0:N], in_=S[P + 2 : P + 3, :, :])
        nc.sync.dma_start(out=gp_strm[P + 1 : P + 2, :, 0:N], in_=S[P + 6 : P + 7, :, :])
        nc.sync.dma_start(out=gp_strm[P : P + 1, :, N : 2 * N], in_=SAN[P + 6 : P + 7, :, :])
        nc.sync.dma_start(out=gp_strm[P + 1 : P + 2, :, N : 2 * N], in_=S[P + 2 : P + 3, :, :])

    # W duplicated along trailing (re|im, j-pair) dims so one DVE op covers
    # both halves of both packed j's
    wt = []
    for mt, (m0, mw) in enumerate(MTILES):
        w = singles.tile([mw, N, 4], F32, tag=f"w{mt}", name=f"w{mt}")
        for half in range(4):
            nc.sync.dma_start(out=w[:, :, half], in_=ins["wmat"][m0 : m0 + mw, :])
        wt.append(w)

    out_sb = singles.tile([96, 96], F32, tag="out_sb", name="out_sb")

    for rep in range(reps):
        out_ps = _emit_compute(tc, parts, singles, stackp, psum_gen, psum_out,
                               mf_stat, mf_strm, gp_stat, gp_strm, wt)

    nc.scalar.copy(out_sb, out_ps[0])
    nc.vector.tensor_add(out_sb, out_sb, out_ps[1])
    nc.sync.dma_start(out=out96, in_=out_sb)


def _emit_compute(tc, parts, singles, stackp, psum_gen, psum_out,
                  mf_stat, mf_strm, gp_stat, gp_strm, wt):
    nc = tc.nc
    out_ps = []
    for mt, (m0, mw) in enumerate(MTILES):
        # fp16 stacks for this m-tile phase; same tag -> phase B reuses slots
        CS = stackp.tile([128, N, 96], F16, tag="cs", name=f"cs{mt}")
        GS = stackp.tile([128, N, 96], F16, tag="gs", name=f"gs{mt}")

        # ---- generate Mf (-> CS, plain copy on ACT) and G' (-> GS, .*W on DVE)
        # One fp32r matmul per j streams [re | im] n-factors (384 cols >= 256
        # keeps fp32r at 1 cycle/row). Two j's share one 2-bank psum tile so a
        # single strided op copies both out (instruction count dominates here).
        for stat, strm, dst, mulw in (
            (mf_stat, mf_strm, CS, False),
            (gp_stat, gp_strm, GS, True),
        ):
            kdim = 8 if stat is mf_stat else 2
            for j0 in range(0, M, 2):
                pt = psum_gen.tile([128, 2, 512], F32, tag="gen", name="pt")
                for dj in range(2):
                    j = j0 + dj
                    a, jj = j % 4, j // 4
                    p0 = 32 * a
                    if "gen" in parts:
                        nc.tensor.matmul(
                            pt[:mw, dj, 0 : 2 * N],
                            stat[p0 : p0 + kdim, jj, m0 : m0 + mw],
                            strm[p0 : p0 + kdim, jj, :],
                            start=True, stop=True,
                            tile_position=(p0, 0),
                        )
                    elif "copy" in parts:
                        nc.vector.memset(pt[:mw, dj, 0 : 2 * N], 0.0)
                if "copy" not in parts:
                    continue
                # src: [mw, n, b(re|im), jpair]; dst: CS/GS cols {j0,j0+1,M+j0,M+j0+1}
                src = pt[:mw, :, 0 : 2 * N].rearrange("p j (b n) -> p n b j", b=2)
                dstv = dst[:mw, :, :].rearrange("p n (b c) -> p n b c", b=2)[:, :, :, j0 : j0 + 2]
                if mulw:
                    nc.vector.tensor_mul(dstv, src, wt[mt].rearrange("p n (b j) -> p n b j", b=2))
                else:
                    nc.scalar.copy(dstv, src)

        # ---- main contraction for this m-tile: accumulate over n (fp16)
        po = psum_out.tile([96, 96], F32, tag="out", name="po")
        out_ps.append(po)
        if "main" in parts:
            for n in range(N):
                nc.tensor.matmul(
                    po, GS[:mw, n, :], CS[:mw, n, :],
                    start=(n == 0), stop=(n == N - 1),
                )
        else:
            nc.vector.memset(po, 0.0)
    return out_ps


_NC = None


def _build_nc(parts=("gen", "copy", "main"), reps=1):
    global _NC
    if _NC is not None and parts == ("gen", "copy", "main") and reps == 1:
        return _NC
    nc = bacc.Bacc("TRN2", target_bir_lowering=False, debug=False)
    ins = {}
    for name, shape in [
        ("mf_stat", [8, M, N]), ("sgn", [128, 1]), ("wmat", [N, N]),
    ]:
        dt = F32R if name == "mf_stat" else F32
        ins[name] = nc.dram_tensor(name, shape, dt, kind="ExternalInput").ap()
    out96 = nc.dram_tensor("out96", [96, 96], F32, kind="ExternalOutput").ap()
    with tile.TileContext(nc) as tc:
        with ExitStack() as ctx:
            _emit(ctx, tc, out96, ins, parts=parts, reps=reps)
    nc.compile()
    if parts == ("gen", "copy", "main") and reps == 1:
        _NC = nc
    return nc


def _host_prep(x, base_re, base_im, beta, idx, pot):
    """f64 host work: scatter, eigh, small terms; returns per-core in_maps
    plus everything needed for final assembly."""
    x = np.asarray(x, np.float64)
    base = np.asarray(base_re, np.float64) + 1j * np.asarray(base_im, np.float64)
    beta = float(np.asarray(beta).reshape(-1)[0])
    idx = np.asarray(idx).astype(np.int64)
    pot = np.asarray(pot, np.float64)

    Bn, Mn = x.shape
    Nn = base.shape[-1]

    JSIG = np.array([[0.0, 1.0], [-1.0, 0.0]], dtype=np.complex128)
    rows = 4 * idx[:, None] + np.arange(2)      # [M,2]
    cols = rows + 2
    H = base.copy()
    top = x[:, :, None, None].astype(np.complex128) * JSIG  # [B,M,2,2]
    bot = np.conj(np.swapaxes(top, -1, -2))
    bi = np.arange(Bn)[:, None, None, None]
    H[bi, rows[None, :, :, None], cols[None, :, None, :]] = top[:, :, :, :]
    H[bi, cols[None, :, :, None], rows[None, :, None, :]] = bot[:, :, :, :]

    L, Q = np.linalg.eigh(H)                    # [B,N], [B,N,N]

    t = np.tanh(0.5 * beta * L)
    dt = 0.5 * beta * (1.0 - t * t)
    q0 = Q[:, 4 * idx + 0, :]
    q1 = Q[:, 4 * idx + 1, :]
    q2 = Q[:, 4 * idx + 2, :]
    q3 = Q[:, 4 * idx + 3, :]
    u, v = q0, q3

    # f (gap equation)
    f = 0.5 * pot[None, :] * np.sum(u * np.conj(v) * t[:, None, :], axis=-1)

    # W: masked tanh divided difference
    D = L[:, None, :] - L[:, :, None]           # D[m,n] = L[n]-L[m]
    mask = np.abs(D) > EPS
    W = np.where(mask, (t[:, None, :] - t[:, :, None]) / np.where(mask, D, 1.0), 0.0)

    # term3 via diag of Mf: dE[j,n] = 2*Re(conj(q0)q3 - conj(q1)q2)[j,n]
    dE = 2.0 * (np.conj(q0) * q3 - np.conj(q1) * q2).real
    y = u * np.conj(v) * dt[:, None, :]
    term3 = np.einsum("Bin,Bjn->Bij", y, dE)

    # per-core device factor tensors (f32); streams/signs derive on-device
    sgn = np.ones((128, 1), np.float32)
    for a in range(4):
        sgn[32 * a + 1 : 32 * a + 8 : 2] = -1.0
    in_maps = []
    for b in range(Bn):
        r = lambda a: np.ascontiguousarray(a.real, np.float32)
        im = lambda a: np.ascontiguousarray(a.imag, np.float32)
        Q0, Q1, Q2, Q3 = q0[b], q1[b], q2[b], q3[b]
        mf_stat = np.stack([r(Q0), r(Q1), r(Q3), r(Q2), im(Q0), im(Q1), im(Q3), im(Q2)])
        in_maps.append({
            "mf_stat": np.ascontiguousarray(mf_stat),
            "sgn": sgn,
            "wmat": np.ascontiguousarray(W[b], dtype=np.float32).astype(np.float32),
        })

    return dict(x=x, pot=pot, f=f, term3=term3, in_maps=in_maps, Bn=Bn, Mn=Mn)


def _assemble(prep, out96_list):
    """Combine device term12 blocks with host terms into (f-x, J-I)."""
    Bn, Mn = prep["Bn"], prep["Mn"]
    term12 = np.empty((Bn, Mn, Mn), dtype=np.complex128)
    for b in range(Bn):
        o = np.asarray(out96_list[b], np.float64)
        re = o[:Mn, :Mn] - o[Mn:, Mn:]
        imp = o[:Mn, Mn:] + o[Mn:, :Mn]
        term12[b] = re + 1j * imp
    J = 0.5 * prep["pot"][None, :, None] * (term12 + prep["term3"])
    out_f = prep["f"] - prep["x"]
    out_J = J - np.eye(Mn, dtype=np.complex128)[None]
    return out_f, out_J


_LAST_RES = None
_RUNNER = None


def _make_runner(nc):
    """Cached replica of bass2jax.run_bass_via_pjrt's multi-core path: build
    the jitted shard_map once and reuse it, so warm calls skip re-tracing."""
    import jax
    import numpy as _np
    from jax.sharding import Mesh, PartitionSpec
    from jax.experimental.shard_map import shard_map
    from concourse import bass2jax, mybir as _mybir

    bass2jax.install_neuronx_cc_hook()
    assert nc.dbg_addr is None
    partition_name = nc.partition_id_tensor.name if nc.partition_id_tensor else None

    in_names, out_names, out_avals, zero_shapes = [], [], [], []
    for alloc in nc.m.functions[0].allocations:
        if not isinstance(alloc, _mybir.MemoryLocationSet):
            continue
        name = alloc.memorylocations[0].name
        if alloc.kind == "ExternalInput":
            if name != partition_name:
                in_names.append(name)
        elif alloc.kind == "ExternalOutput":
            shape = tuple(alloc.tensor_shape)
            dtype = _mybir.dt.np(alloc.dtype)
            out_names.append(name)
            out_avals.append(jax.core.ShapedArray(shape, dtype))
            zero_shapes.append((shape, dtype))
    n_params, n_outs = len(in_names), len(out_avals)
    all_names = in_names + out_names
    if partition_name is not None:
        all_names = all_names + [partition_name]

    def _body(*args):
        operands = list(args)
        if partition_name is not None:
            operands.append(bass2jax.partition_id_tensor())
        outs = bass2jax._bass_exec_p.bind(
            *operands,
            out_avals=tuple(out_avals),
            in_names=tuple(all_names),
            out_names=tuple(out_names),
            lowering_input_output_aliases=(),
            sim_require_finite=True,
            sim_require_nnan=True,
            nc=nc,
        )
        return tuple(outs)

    devices = jax.devices()[:B]
    mesh = Mesh(_np.asarray(devices), ("core",))
    specs = (PartitionSpec("core"),) * (n_params + n_outs)
    sharded = jax.jit(
        shard_map(_body, mesh=mesh, in_specs=specs,
                  out_specs=(PartitionSpec("core"),) * n_outs, check_rep=False),
        donate_argnums=tuple(range(n_params, n_params + n_outs)),
        keep_unused=True,
    )

    def run(in_maps):
        concat_in = [
            _np.concatenate([_np.asarray(in_maps[c][nm]) for c in range(B)], axis=0)
            for nm in in_names
        ]
        concat_zeros = [
            _np.zeros((B * s[0], *s[1:]), dt) for (s, dt) in zero_shapes
        ]
        out_arrs = sharded(*concat_in, *concat_zeros)
        return [
            {nm: _np.asarray(out_arrs[i]).reshape(B, *out_avals[i].shape)[c]
             for i, nm in enumerate(out_names)}
            for c in range(B)
        ]

    return run


def kernel(x, base_re, base_im, beta, idx, pot):
    global _LAST_RES, _RUNNER
    prep = _host_prep(x, base_re, base_im, beta, idx, pot)
    assert prep["Bn"] == B and prep["Mn"] == M

    nc = _build_nc()
    if _RUNNER is None:
        _RUNNER = _make_runner(nc)
    results = _RUNNER(prep["in_maps"])
    out96_list = [results[b]["out96"] for b in range(B)]
    return _assemble(prep, out96_list)
